# revision 8
# baseline (speedup 1.0000x reference)
"""DepthToSpace (cell=4, 4 split groups) Trainium2 Bass kernel.

Full input x: [8, 64, 256, 256] f32 -> output [8, 4, 1024, 1024] f32.
out[b, s, 4h+r, 4w+c] = x[b, 16s + 4r + c, h, w]

Sharding: data parallel over batch — core b handles x[b] (16.8 MB in/out).

Per-core plan (pure data movement, memory-bound):
  Partition p = cg*64 + h4, where cg = r//2 (channel subgroup) and
  h4 = h//4. Per split group s:
    load   : 2 DMAs (one per cg, complementary partition halves)
             x[16s+8cg+chl, 4h4+h4b, w] -> X[64cg+h4, chl, h4b, w]
             (4KB contiguous DRAM runs per (partition, chl))
    shuffle: Y[p, h4b, rl, w, c] = X[p, 4rl+c, h4b, w]
             rl=0 on the vector engine, rl=1 on the scalar engine
    store  : 2 DMAs (one per cg)
             Y -> y[s] rows 16h4+4h4b+2cg+rl, cols 4w+c
             (8KB contiguous DRAM runs)
X is triple buffered so loads queue eagerly; Y is double buffered.
The 4-byte-granularity interleave happens on-chip where strided access
is cheap; both DMA directions keep multi-KB contiguous runs.
"""

import sys

sys.path.insert(0, "/opt/trn_rl_repo")

import numpy as np

import concourse.bass as bass
import concourse.mybir as mybir
from concourse.bass_utils import run_bass_kernel_spmd

B, C, H, W = 8, 64, 256, 256
S = 4
CELL = 4  # sqrt(C // S)
CPG = C // S  # channels per group = CELL^2 = 16
P = 128  # SBUF partitions
N_CORES = 8

CG = 2  # channel subgroups per split group (r halves)
CHL = CPG // CG  # channels per subgroup = 8
H4 = 64  # h blocks (partition index within subgroup)
H4B = H // H4  # h rows per partition = 4
RL = CELL // CG  # r values per subgroup = 2

NXB = 3  # X buffers
NYB = 2  # Y buffers


def build_program():
    nc = bass.Bass()
    x = nc.declare_dram_parameter("x", [C, H, W], mybir.dt.float32, isOutput=False)
    y = nc.declare_dram_parameter(
        "y", [S, H * CELL, W * CELL], mybir.dt.float32, isOutput=True
    )

    with (
        nc.sbuf_tensor([P, CHL, H4B, W], mybir.dt.float32) as X0,
        nc.sbuf_tensor([P, CHL, H4B, W], mybir.dt.float32) as X1,
        nc.sbuf_tensor([P, CHL, H4B, W], mybir.dt.float32) as X2,
        nc.sbuf_tensor([P, H4B, RL, W, CELL], mybir.dt.float32) as Y0,
        nc.sbuf_tensor([P, H4B, RL, W, CELL], mybir.dt.float32) as Y1,
        nc.semaphore("in00") as in00,
        nc.semaphore("in01") as in01,
        nc.semaphore("in10") as in10,
        nc.semaphore("in11") as in11,
        nc.semaphore("in20") as in20,
        nc.semaphore("in21") as in21,
        nc.semaphore("out00") as out00,
        nc.semaphore("out01") as out01,
        nc.semaphore("out10") as out10,
        nc.semaphore("out11") as out11,
        nc.semaphore("shuf_v") as shuf_v,
        nc.semaphore("shuf_a") as shuf_a,
        nc.Block() as block,
    ):
        Xt = [X0, X1, X2]
        Yt = [Y0, Y1]
        in_sem = [[in00, in01], [in10, in11], [in20, in21]]
        out_sem = [[out00, out01], [out10, out11]]

        def load_ap(s, cg):
            # DRAM side: per (partition, chl) a 4KB contiguous run
            return x[s * CPG + cg * CHL : s * CPG + (cg + 1) * CHL].rearrange(
                "chl (h4 h4b) w -> h4 chl h4b w", h4=H4
            )

        def store_ap(s, cg):
            # y[s] rows 16h4+4h4b+2cg+rl, cols 4w+c for this cg's partitions
            ap = y[s].rearrange(
                "(h4 h4b cg rl) (w c) -> cg h4 h4b rl w c",
                h4=H4,
                h4b=H4B,
                cg=CG,
                rl=RL,
                c=CELL,
            )
            return ap[cg]

        def shuffle_aps(s, rl):
            # src: [p, c, h4b, w]; dst: same iteration order (p, c, h4b, w)
            src = Xt[s % NXB][:, rl * CELL : (rl + 1) * CELL, :, :]
            dst = Yt[s % NYB][:, :, rl, :, :].transpose([0, 3, 1, 2])
            return src, dst

        @block.sync
        def _(sync):
            for s in range(S):
                if s >= NXB:
                    # X[s%NXB] is free once both shuffle halves of s-NXB are done
                    sync.wait_ge(shuf_v, s - NXB + 1)
                    sync.wait_ge(shuf_a, s - NXB + 1)
                for cg in range(CG):
                    sync.dma_start(
                        out=Xt[s % NXB][cg * H4 : (cg + 1) * H4], in_=load_ap(s, cg)
                    ).then_inc(in_sem[s % NXB][cg], 16)

        @block.vector
        def _(vector):
            for s in range(S):
                for cg in range(CG):
                    vector.wait_ge(in_sem[s % NXB][cg], 16 * (s // NXB + 1))
                if s >= NYB:
                    # Y[s%NYB] is free once both store halves of s-NYB are done
                    for cg in range(CG):
                        vector.wait_ge(out_sem[s % NYB][cg], 16 * (s // NYB))
                src, dst = shuffle_aps(s, 0)
                vector.tensor_copy(out=dst, in_=src).then_inc(shuf_v, 1)

        @block.scalar
        def _(scalar):
            for s in range(S):
                for cg in range(CG):
                    scalar.wait_ge(in_sem[s % NXB][cg], 16 * (s // NXB + 1))
                if s >= NYB:
                    for cg in range(CG):
                        scalar.wait_ge(out_sem[s % NYB][cg], 16 * (s // NYB))
                src, dst = shuffle_aps(s, 1)
                scalar.copy(out=dst, in_=src).then_inc(shuf_a, 1)
                scalar.wait_ge(shuf_v, s + 1)
                scalar.wait_ge(shuf_a, s + 1)
                for cg in range(CG):
                    scalar.dma_start(
                        out=store_ap(s, cg), in_=Yt[s % NYB][cg * H4 : (cg + 1) * H4]
                    ).then_inc(out_sem[s % NYB][cg], 16)
            for b in range(NYB):
                for cg in range(CG):
                    scalar.wait_ge(out_sem[b][cg], 16 * (S // NYB))

    return nc


def run_sharded(x: np.ndarray, trace: bool = False):
    """Shard x over batch across 8 cores, run, gather. Returns (out, results)."""
    assert x.shape == (B, C, H, W), x.shape
    nc = build_program()
    in_maps = [{"x": np.ascontiguousarray(x[b])} for b in range(N_CORES)]
    res = run_bass_kernel_spmd(nc, in_maps, list(range(N_CORES)), trace=trace)
    out = np.stack([res.results[b]["y"] for b in range(N_CORES)], axis=0)
    return out.astype(x.dtype, copy=False), res


def kernel(**inputs: np.ndarray) -> np.ndarray:
    x = np.asarray(inputs["x"], dtype=np.float32)
    out, _ = run_sharded(x, trace=False)
    return out


# revision 11
# speedup vs baseline: 1.2518x; 1.2518x over previous
"""DepthToSpace (cell=4, 4 split groups) Trainium2 Bass kernel.

Full input x: [8, 64, 256, 256] f32 -> output [8, 4, 1024, 1024] f32.
out[b, s, 4h+r, 4w+c] = x[b, 16s + 4r + c, h, w]

Sharding: data parallel over batch — core b handles x[b] (16.8 MB in/out).

Per-core plan (pure data movement, memory-bound): partition p = h//2.
Work is split into two independent lanes by r-half (rl = r//2), each
processing 2 MB units per split group s:
  lane V (rl=0): load x[16s .. 16s+8]   -> shuffle on VectorE -> store
  lane A (rl=1): load x[16s+8 .. 16s+16] -> shuffle on ScalarE -> store
  load   : X[p, chl, h2, w] = x[16s+8rl+chl, 2p+h2, w]  (2KB runs)
  shuffle: Y[p, h2, r2, w, c] = X[p, 4r2+c, h2, w]      (strided copy)
  store  : Y -> y[s] rows 8p+4h2+2rl+r2, cols 4w+c      (8KB runs)
Lane V stores issue from GPSIMD (SWDGE), lane A stores from ScalarE
(HWDGE), loads from Sync (HWDGE) — three DMA issue paths, two compute
engines, no cross-lane dependencies. X is triple buffered so loads
queue eagerly; Y is double buffered.
"""

import sys

sys.path.insert(0, "/opt/trn_rl_repo")

import numpy as np

import concourse.bass as bass
import concourse.mybir as mybir
from concourse.bass_utils import run_bass_kernel_spmd

B, C, H, W = 8, 64, 256, 256
S = 4
CELL = 4  # sqrt(C // S)
CPG = C // S  # channels per group = 16
P = 128  # SBUF partitions
HB = H // P  # h rows per partition = 2
N_CORES = 8

RL = 2  # lanes (r halves)
R2 = CELL // RL  # r values per lane = 2
CHL = CPG // RL  # channels per lane = 8

NXB = 3  # X buffers per lane
NYB = 2  # Y buffers per lane


def build_program():
    nc = bass.Bass()
    x = nc.declare_dram_parameter("x", [C, H, W], mybir.dt.float32, isOutput=False)
    y = nc.declare_dram_parameter(
        "y", [S, H * CELL, W * CELL], mybir.dt.float32, isOutput=True
    )

    from contextlib import ExitStack

    with ExitStack() as ctx:
        sb = lambda name, shape: ctx.enter_context(
            nc.sbuf_tensor(name, shape, mybir.dt.float32)
        )
        sem = lambda name: ctx.enter_context(nc.semaphore(name))
        Xv = [sb(f"XV{i}", [P, CHL, HB, W]) for i in range(NXB)]
        Xa = [sb(f"XA{i}", [P, CHL, HB, W]) for i in range(NXB)]
        Yv = [sb(f"YV{i}", [P, HB, R2, W, CELL]) for i in range(NYB)]
        Ya = [sb(f"YA{i}", [P, HB, R2, W, CELL]) for i in range(NYB)]
        inv = [sem(f"inv{i}") for i in range(NXB)]
        ina = [sem(f"ina{i}") for i in range(NXB)]
        outv = [sem(f"outv{i}") for i in range(NYB)]
        outa = [sem(f"outa{i}") for i in range(NYB)]
        shuf_v = sem("shuf_v")
        shuf_a = sem("shuf_a")
        outv0, outv1 = outv
        outa0, outa1 = outa
        block = ctx.enter_context(nc.Block())

        def load_ap(s, rl):
            # x channels for (s, rl); DRAM runs of 2KB per (p, chl)
            c0 = s * CPG + rl * CHL
            return x[c0 : c0 + CHL].rearrange("chl (p h2) w -> p chl h2 w", h2=HB)

        def store_ap(s, rl):
            # y[s] rows 8p+4h2+2rl+r2, cols 4w+c -> 8KB contiguous runs
            ap = y[s].rearrange(
                "(p h2 rl r2) (w c) -> rl p h2 r2 w c", h2=HB, rl=RL, r2=R2, c=CELL
            )
            return ap[rl]

        def copy_aps(Xb, Yb, h2):
            # src [p, r2, c, w] == dst iteration (p, r2, c, w)
            src = Xb[:].rearrange("p (r2 c) h2 w -> p r2 c h2 w", r2=R2)[:, :, :, h2, :]
            dst = Yb[:, h2].transpose([0, 1, 3, 2])
            return src, dst

        @block.sync
        def _(sync):
            for s in range(S):
                if s >= NXB:
                    sync.wait_ge(shuf_v, HB * (s - NXB + 1))
                sync.dma_start(out=Xv[s % NXB][:], in_=load_ap(s, 0)).then_inc(
                    inv[s % NXB], 16
                )
                if s >= NXB:
                    sync.wait_ge(shuf_a, HB * (s - NXB + 1))
                sync.dma_start(out=Xa[s % NXB][:], in_=load_ap(s, 1)).then_inc(
                    ina[s % NXB], 16
                )

        @block.vector
        def _(vector):
            for s in range(S):
                vector.wait_ge(inv[s % NXB], 16 * (s // NXB + 1))
                if s >= NYB:
                    vector.wait_ge(outv[s % NYB], 16 * (s // NYB))
                for h2 in range(HB):
                    src, dst = copy_aps(Xv[s % NXB], Yv[s % NYB], h2)
                    vector.tensor_copy(out=dst, in_=src).then_inc(shuf_v, 1)

        @block.scalar
        def _(scalar):
            for s in range(S):
                scalar.wait_ge(ina[s % NXB], 16 * (s // NXB + 1))
                if s >= NYB:
                    scalar.wait_ge(outa[s % NYB], 16 * (s // NYB))
                for h2 in range(HB):
                    src, dst = copy_aps(Xa[s % NXB], Ya[s % NYB], h2)
                    scalar.copy(out=dst, in_=src).then_inc(shuf_a, 1)
                scalar.wait_ge(shuf_a, HB * (s + 1))
                scalar.dma_start(out=store_ap(s, 1), in_=Ya[s % NYB][:]).then_inc(
                    outa[s % NYB], 16
                )
            scalar.wait_ge(outa0, 16 * (S // NYB))
            scalar.wait_ge(outa1, 16 * (S // NYB))

        @block.gpsimd
        def _(gpsimd):
            for s in range(S):
                gpsimd.wait_ge(shuf_v, HB * (s + 1))
                gpsimd.dma_start(out=store_ap(s, 0), in_=Yv[s % NYB][:]).then_inc(
                    outv[s % NYB], 16
                )
            gpsimd.wait_ge(outv0, 16 * (S // NYB))
            gpsimd.wait_ge(outv1, 16 * (S // NYB))

    return nc


def run_sharded(x: np.ndarray, trace: bool = False):
    """Shard x over batch across 8 cores, run, gather. Returns (out, results)."""
    assert x.shape == (B, C, H, W), x.shape
    nc = build_program()
    in_maps = [{"x": np.ascontiguousarray(x[b])} for b in range(N_CORES)]
    res = run_bass_kernel_spmd(nc, in_maps, list(range(N_CORES)), trace=trace)
    out = np.stack([res.results[b]["y"] for b in range(N_CORES)], axis=0)
    return out.astype(x.dtype, copy=False), res


def kernel(**inputs: np.ndarray) -> np.ndarray:
    x = np.asarray(inputs["x"], dtype=np.float32)
    out, _ = run_sharded(x, trace=False)
    return out


# revision 13
# speedup vs baseline: 1.2557x; 1.0031x over previous
"""DepthToSpace (cell=4, 4 split groups) Trainium2 Bass kernel.

Full input x: [8, 64, 256, 256] f32 -> output [8, 4, 1024, 1024] f32.
out[b, s, 4h+r, 4w+c] = x[b, 16s + 4r + c, h, w]

Sharding: data parallel over batch — core b handles x[b] (16.8 MB in/out).

Per-core plan (pure data movement, memory-bound): partition p = h//2.
Loads are split into two lanes by r-half (rl = r//2, 8 channels each)
and issued on two DMA paths so loads take ~2/3 of SDMA bandwidth and
the last load lands early (hiding the shuffle+store tail):
  lane V (rl=0): load via Sync (HWDGE)  -> shuffle on VectorE
  lane A (rl=1): load via GPSIMD (SWDGE) -> shuffle on ScalarE
  load   : Xl[p, chl, h2, w] = x[16s+8rl+chl, 2p+h2, w]  (2KB runs)
  shuffle: Y[p, h2, 2rl+r2, w, c] = Xl[p, 4r2+c, h2, w]  (strided copy)
  store  : one DMA per s from ScalarE: Y -> y[s] rows 8p+4h2+r,
           cols 4w+c — a single fully contiguous 4MB region (32KB runs)
X is triple buffered per lane so loads queue eagerly; Y (shared by both
lanes; disjoint r halves) is double buffered. The 4-byte-granularity
interleave happens on-chip where strided access is cheap.
"""

import sys

sys.path.insert(0, "/opt/trn_rl_repo")

import numpy as np

import concourse.bass as bass
import concourse.mybir as mybir
from concourse.bass_utils import run_bass_kernel_spmd

B, C, H, W = 8, 64, 256, 256
S = 4
CELL = 4  # sqrt(C // S)
CPG = C // S  # channels per group = 16
P = 128  # SBUF partitions
HB = H // P  # h rows per partition = 2
N_CORES = 8

RL = 2  # lanes (r halves)
R2 = CELL // RL  # r values per lane = 2
CHL = CPG // RL  # channels per lane = 8

NXB = 3  # X buffers per lane
NYB = 2  # Y buffers (shared)


def build_program():
    nc = bass.Bass()
    x = nc.declare_dram_parameter("x", [C, H, W], mybir.dt.float32, isOutput=False)
    y = nc.declare_dram_parameter(
        "y", [S, H * CELL, W * CELL], mybir.dt.float32, isOutput=True
    )

    from contextlib import ExitStack

    with ExitStack() as ctx:
        sb = lambda name, shape: ctx.enter_context(
            nc.sbuf_tensor(name, shape, mybir.dt.float32)
        )
        sem = lambda name: ctx.enter_context(nc.semaphore(name))
        Xv = [sb(f"XV{i}", [P, CHL, HB, W]) for i in range(NXB)]
        Xa = [sb(f"XA{i}", [P, CHL, HB, W]) for i in range(NXB)]
        Yt = [sb(f"Y{i}", [P, HB, CELL, W, CELL]) for i in range(NYB)]
        inv = [sem(f"inv{i}") for i in range(NXB)]
        ina = [sem(f"ina{i}") for i in range(NXB)]
        outs = [sem(f"outs{i}") for i in range(NYB)]
        shuf_v = sem("shuf_v")
        shuf_a = sem("shuf_a")
        block = ctx.enter_context(nc.Block())

        def load_ap(s, rl):
            # x channels for (s, rl); DRAM runs of 2KB per (p, chl)
            c0 = s * CPG + rl * CHL
            return x[c0 : c0 + CHL].rearrange("chl (p h2) w -> p chl h2 w", h2=HB)

        def store_ap(s):
            # y[s] as [p, h2, r, w, c]: row = 8p+4h2+r, col = 4w+c.
            # Fully contiguous: 32KB per partition, one 4MB region.
            return y[s].rearrange(
                "(p h2 r) (w c) -> p h2 r w c", h2=HB, r=CELL, c=CELL
            )

        def copy_aps(Xb, Yb, rl, h2):
            # src [p, r2, c, w] == dst iteration (p, r2, c, w)
            src = Xb[:].rearrange("p (r2 c) h2 w -> p r2 c h2 w", r2=R2)[:, :, :, h2, :]
            dst = Yb[:, h2, rl * R2 : (rl + 1) * R2].transpose([0, 1, 3, 2])
            return src, dst

        @block.sync
        def _(sync):
            for s in range(S):
                if s >= NXB:
                    sync.wait_ge(shuf_v, HB * (s - NXB + 1))
                sync.dma_start(out=Xv[s % NXB][:], in_=load_ap(s, 0)).then_inc(
                    inv[s % NXB], 16
                )

        @block.gpsimd
        def _(gpsimd):
            for s in range(S):
                if s >= NXB:
                    gpsimd.wait_ge(shuf_a, HB * (s - NXB + 1))
                gpsimd.dma_start(out=Xa[s % NXB][:], in_=load_ap(s, 1)).then_inc(
                    ina[s % NXB], 16
                )

        @block.vector
        def _(vector):
            for s in range(S):
                vector.wait_ge(inv[s % NXB], 16 * (s // NXB + 1))
                if s >= NYB:
                    vector.wait_ge(outs[s % NYB], 16 * (s // NYB))
                for h2 in range(HB):
                    src, dst = copy_aps(Xv[s % NXB], Yt[s % NYB], 0, h2)
                    vector.tensor_copy(out=dst, in_=src).then_inc(shuf_v, 1)

        @block.scalar
        def _(scalar):
            for s in range(S):
                scalar.wait_ge(ina[s % NXB], 16 * (s // NXB + 1))
                if s >= NYB:
                    scalar.wait_ge(outs[s % NYB], 16 * (s // NYB))
                for h2 in range(HB):
                    src, dst = copy_aps(Xa[s % NXB], Yt[s % NYB], 1, h2)
                    scalar.copy(out=dst, in_=src).then_inc(shuf_a, 1)
                scalar.wait_ge(shuf_v, HB * (s + 1))
                scalar.wait_ge(shuf_a, HB * (s + 1))
                scalar.dma_start(out=store_ap(s), in_=Yt[s % NYB][:]).then_inc(
                    outs[s % NYB], 16
                )
            scalar.wait_ge(outs[0], 16 * (S // NYB))
            scalar.wait_ge(outs[1], 16 * (S // NYB))

    return nc


def run_sharded(x: np.ndarray, trace: bool = False):
    """Shard x over batch across 8 cores, run, gather. Returns (out, results)."""
    assert x.shape == (B, C, H, W), x.shape
    nc = build_program()
    in_maps = [{"x": np.ascontiguousarray(x[b])} for b in range(N_CORES)]
    res = run_bass_kernel_spmd(nc, in_maps, list(range(N_CORES)), trace=trace)
    out = np.stack([res.results[b]["y"] for b in range(N_CORES)], axis=0)
    return out.astype(x.dtype, copy=False), res


def kernel(**inputs: np.ndarray) -> np.ndarray:
    x = np.asarray(inputs["x"], dtype=np.float32)
    out, _ = run_sharded(x, trace=False)
    return out
